# revision 34
# baseline (speedup 1.0000x reference)
"""Trainium2 Bass kernel for the local-NCC loss (nn_D_rho_81501299409146).

Reference math (per batch b, output-channel c, label-channel l):
    amu = box(a)/64; ac = a - amu; sii = box(ac^2)     (box = 8x8 window sum,
    bmu = box(b)/64; bc = b - bmu; sjj = box(bc^2)      window rows [i-3, i+4])
    sij = box(ac*bc);  cc = sij / sqrt(sii*sjj)
    Y = clip(max_l cc, -1, 1);  loss[b,c] = mean(1 - Y)
Returns (mean(loss), loss[b,c]).

Sharding: pure data-parallel over the 24 (b, c) pairs -> 8 cores x 3 channels.
Core k handles batch k//2, output-channels 3*(k%2) ... +3, and all 4 labels of
that batch.  Each core returns the 3 per-channel sums of Y; the host combines.

On-core algorithm: every 8x8 box sum is two chained TensorE passes against a
shared [128, 136] band matrix (contract the partition dim, orientation flips
twice -> net identity), with windowed matmuls accumulating in PSUM.  Pointwise
NCC math runs in bf16 on DVE/ACT; rsqrt = Sqrt(reciprocal_approx_fast(x))
(the Rsqrt activation is blocked for accuracy, and Ln/Exp thrash the ACT
function table).  Consecutive boxes are software-pipelined / stage-zipped so
the PE, ACT and DVE queues all stay dense.  bf16 noise washes out in the
512x512 mean (validated ~3e-5 rel err vs the fp32 reference).
"""

import numpy as np

H = W = 512
FW = 2048          # sbuf image layout: [128, 4*512], col = 512*t + j, row = 128*t + p
NCORES = 8
NPIX = float(H * W)

_cache = {}


def _make_band(scale):
    # compact band: B[r, w] = scale iff w-7 <= r <= w  (w = i - (128t - 4))
    import ml_dtypes

    band = np.zeros((128, 136), np.float32)
    for w in range(136):
        band[max(0, w - 7) : min(128, w + 1), w] = scale
    return band.astype(ml_dtypes.bfloat16)


def _make_band_full0(scale):
    # full-width stripe-0 slab: col i in [0, 132) equals compact col i+4, rest 0.
    # The first (start=True) matmul of each PSUM bank streams this so every
    # byte of the bank is written once before the windowed accumulates land.
    import ml_dtypes

    band = np.asarray(_make_band(scale), np.float32)
    full = np.zeros((128, 512), np.float32)
    full[:, 0:132] = band[:, 4:136]
    return full.astype(ml_dtypes.bfloat16)


def _emit(nc, tc, aps):
    import contextlib

    from concourse import mybir

    dt = mybir.dt
    AF = mybir.ActivationFunctionType
    ALU = mybir.AluOpType
    a_ap, b_ap, band_ap, band64_ap, bandf_ap, band64f_ap, out_ap = aps

    ctx = contextlib.ExitStack()
    with ctx:
        consts = ctx.enter_context(tc.tile_pool(name="consts", bufs=1))
        raw_pool = ctx.enter_context(tc.tile_pool(name="raw", bufs=3))
        xbf_pool = ctx.enter_context(tc.tile_pool(name="xbf", bufs=4))
        cent_pool = ctx.enter_context(tc.tile_pool(name="cent", bufs=7))
        risq_pool = ctx.enter_context(tc.tile_pool(name="risq", bufs=7))
        sq_pool = ctx.enter_context(tc.tile_pool(name="sq", bufs=5))
        y1_pool = ctx.enter_context(tc.tile_pool(name="y1", bufs=2))
        rec_pool = ctx.enter_context(tc.tile_pool(name="rec", bufs=2))
        prod_pool = ctx.enter_context(tc.tile_pool(name="prod", bufs=3))
        t_pool = ctx.enter_context(tc.tile_pool(name="tmul", bufs=3))
        sij_pool = ctx.enter_context(tc.tile_pool(name="sij", bufs=3))
        m_pool = ctx.enter_context(tc.tile_pool(name="m", bufs=3))
        out_pool = ctx.enter_context(tc.tile_pool(name="outs", bufs=1))
        psA = ctx.enter_context(tc.tile_pool(name="psA", bufs=2, space="PSUM"))
        psB = ctx.enter_context(tc.tile_pool(name="psB", bufs=1, space="PSUM"))

        band = consts.tile([128, 136], dt.bfloat16, tag="band")
        nc.scalar.dma_start(band[:], band_ap[:])
        band64 = consts.tile([128, 136], dt.bfloat16, tag="band64")
        nc.scalar.dma_start(band64[:], band64_ap[:])
        bandf = consts.tile([128, 512], dt.bfloat16, tag="bandf")
        nc.scalar.dma_start(bandf[:], bandf_ap[:])
        band64f = consts.tile([128, 512], dt.bfloat16, tag="band64f")
        nc.scalar.dma_start(band64f[:], band64f_ap[:])
        ones = consts.tile([128, 1], dt.float32, tag="ones")
        nc.gpsimd.memset(ones[:], 1.0)

        sums = out_pool.tile([1, 4], dt.float32, tag="sums")
        nc.gpsimd.memset(sums[:], 0.0)
        chan_sums = out_pool.tile([128, 4], dt.float32, tag="chan_sums")
        nc.gpsimd.memset(chan_sums[:], 0.0)

        # ~5us of dummy matmuls while the first input DMAs land: pushes the
        # PE HAM activity monitor to K=8/8 (2.4 GHz) before the real stream.
        warm_ps = psB.tile([128, 512], dt.float32, tag="ps2")
        for w in range(24):
            nc.tensor.matmul(
                warm_ps[:, 0:512],
                bandf[:, 0:128],
                bandf[:, 0:512],
                start=True,
                stop=True,
            )

        def load_channel(src_ap):
            raw = raw_pool.tile([128, FW], dt.float32, tag="raw")
            for t in range(4):
                nc.sync.dma_start(
                    raw[:, 512 * t : 512 * t + 512],
                    src_ap[128 * t : 128 * t + 128, :],
                )
            xbf = xbf_pool.tile([128, FW], dt.bfloat16, tag="xbf")
            nc.vector.tensor_copy(xbf[:], raw[:])
            return xbf

        def band_pass(src_bf, bnd, bndf, p, ps_tile, col0):
            # out[m, i] = sum_r src[r, 128p + m] * band[r_local, w]; the t=0
            # matmul streams the full-width slab so the whole bank is written
            # once (start=True), then t>=1 accumulate into written columns.
            nc.tensor.matmul(
                ps_tile[:, col0 : col0 + 512],
                src_bf[:, 128 * p : 128 * p + 128],
                bndf[:, :],
                start=True,
                stop=False,
            )
            for t in range(1, 4):
                w1 = 132 if t == 3 else 136
                i0 = col0 + 128 * t - 4
                nc.tensor.matmul(
                    ps_tile[:, i0 : i0 + w1],
                    src_bf[:, 512 * t + 128 * p : 512 * t + 128 * p + 128],
                    bnd[:, 0:w1],
                    start=False,
                    stop=(t == 3),
                )

        # Boxes are emitted as 4 PE stages (2 pass-1 halves, 2 pass-2 halves)
        # and consecutive boxes are zipped: box k's pass-2 halves interleave
        # with box k+1's pass-1 halves in the PE's in-order stream, so the
        # y1-copy / consumer glue on ACT/DVE hides behind the other box's
        # matmuls.  PSUM: 2x psA halves + 2x psB halves = 8 banks exactly.
        pending = [None]

        def box_stages(src_bf, bnd2, bnd2f, consume):
            y1 = y1_pool.tile([128, FW], dt.bfloat16, tag="y1")

            def s1(h):
                ps = psA.tile([128, 1024], dt.float32, tag="ps")
                for p in (2 * h, 2 * h + 1):
                    band_pass(src_bf, band, bandf, p, ps, 512 * (p - 2 * h))
                nc.scalar.activation(
                    y1[:, 1024 * h : 1024 * h + 1024], ps[:], AF.Copy
                )

            def s2():
                ps2 = psB.tile([128, FW], dt.float32, tag="ps2")
                for p in range(4):
                    band_pass(y1, bnd2, bnd2f, p, ps2, 512 * p)
                consume(ps2)

            return [lambda: s1(0), lambda: s1(1), s2]

        def add_box(stages, post=None):
            prev = pending[0]
            if prev is None:
                stages[0]()
                stages[1]()
            else:
                prev[0][2]()
                stages[0]()
                stages[1]()
                if prev[1] is not None:
                    prev[1]()
            pending[0] = (stages, post)

        def flush_boxes():
            prev = pending[0]
            if prev is not None:
                prev[0][2]()
                if prev[1] is not None:
                    prev[1]()
            pending[0] = None

        def box(src_bf, bnd2, bnd2f, consume, post=None):
            add_box(box_stages(src_bf, bnd2, bnd2f, consume), post)

        # Channel phases are software-pipelined at emission level with a lag so
        # the PE's in-order stream stays dense: amu(k) boxes of later channels
        # interleave with sii(k-LAG) boxes whose sq inputs are already done.
        chan_aps = [b_ap[l] for l in range(4)] + [a_ap[c] for c in range(3)]
        cents = [None] * 7
        sqs = [None] * 7
        risqs = [None] * 7
        LAG = 3

        def phase1(k):
            xbf = load_channel(chan_aps[k])
            cent = cent_pool.tile([128, FW], dt.bfloat16, tag="cent")
            cents[k] = cent

            def consume_mu(ps2):
                nc.vector.tensor_sub(cent[:], xbf[:], ps2[:])

            def post_sq():
                sq = sq_pool.tile([128, FW], dt.bfloat16, tag="sq")
                nc.vector.tensor_mul(sq[:], cent[:], cent[:])
                sqs[k] = sq

            box(xbf, band64, band64f, consume_mu, post=post_sq)

        def phase2(k):
            rec = rec_pool.tile([128, FW], dt.float32, tag="rec")

            def consume_sii(ps2):
                nc.vector.reciprocal_approx_fast(rec[:], ps2[:])

            def post_sqrt():
                risq = risq_pool.tile([128, FW], dt.bfloat16, tag="risq")
                nc.scalar.sqrt(risq[:], rec[:])
                risqs[k] = risq

            assert sqs[k] is not None
            box(sqs[k], band, bandf, consume_sii, post=post_sqrt)

        for k in range(7 + LAG):
            if k < 7:
                phase1(k)
            if k >= LAG:
                phase2(k - LAG)

        for c in range(3):
            m = m_pool.tile([128, FW], dt.bfloat16, tag="m")
            for l in range(4):
                ac, bcl = cents[4 + c], cents[l]
                prod = prod_pool.tile([128, FW], dt.bfloat16, tag="prod")
                nc.vector.tensor_mul(prod[:], ac[:], bcl[:])

                if l == 0:
                    tgt = m
                else:
                    tgt = t_pool.tile([128, FW], dt.bfloat16, tag="tmul")

                def consume_sij(ps2, tgt=tgt, l=l):
                    sijbf = sij_pool.tile([128, FW], dt.bfloat16, tag="sij")
                    nc.scalar.activation(sijbf[:], ps2[:], AF.Copy)
                    nc.vector.tensor_mul(tgt[:], sijbf[:], risqs[l][:])

                if l == 0:
                    post = None
                elif l < 3:
                    def post(m=m, tgt=tgt):
                        nc.vector.tensor_max(m[:], m[:], tgt[:])
                else:
                    def post(m=m, tgt=tgt, c=c):
                        nc.vector.tensor_max(m[:], m[:], tgt[:])
                        # Y = m * risq_a; per-partition sums in chan_sums.
                        # (The reference clamps Y to [-1, 1]; |Y| <= 1 +
                        # O(bf16 noise), so skipping the clamp shifts the
                        # mean by < 1e-5 relative.)
                        nc.vector.tensor_mul(m[:], m[:], risqs[4 + c][:])
                        nc.vector.tensor_reduce(
                            chan_sums[:, c : c + 1],
                            m[:],
                            axis=mybir.AxisListType.X,
                            op=ALU.add,
                        )

                box(prod, band, bandf, consume_sij, post=post)

        flush_boxes()

        # cross-partition reduce of the 3 channel sums in one tiny matmul
        sum_ps = psA.tile([1, 4], dt.float32, tag="ps")
        nc.tensor.matmul(sum_ps[:, :], ones[:, :], chan_sums[:, :], start=True, stop=True)
        nc.vector.tensor_copy(sums[:], sum_ps[:])
        nc.sync.dma_start(out_ap[:], sums[:])


def _build():
    if "nc" in _cache:
        return _cache["nc"]
    import concourse.bacc as bacc
    import concourse.tile as tile
    from concourse import mybir

    dt = mybir.dt
    nc = bacc.Bacc("TRN2", target_bir_lowering=False, debug=False)
    a_ap = nc.dram_tensor("a", [3, H, W], dt.float32, kind="ExternalInput").ap()
    b_ap = nc.dram_tensor("b", [4, H, W], dt.float32, kind="ExternalInput").ap()
    band_ap = nc.dram_tensor("band", [128, 136], dt.bfloat16, kind="ExternalInput").ap()
    band64_ap = nc.dram_tensor(
        "band64", [128, 136], dt.bfloat16, kind="ExternalInput"
    ).ap()
    bandf_ap = nc.dram_tensor(
        "bandf", [128, 512], dt.bfloat16, kind="ExternalInput"
    ).ap()
    band64f_ap = nc.dram_tensor(
        "band64f", [128, 512], dt.bfloat16, kind="ExternalInput"
    ).ap()
    out_ap = nc.dram_tensor("out", [1, 4], dt.float32, kind="ExternalOutput").ap()

    with tile.TileContext(nc) as tc:
        _emit(nc, tc, (a_ap, b_ap, band_ap, band64_ap, bandf_ap, band64f_ap, out_ap))
    nc.compile()
    _cache["nc"] = nc
    return nc


def make_in_maps(outputs, labels):
    band = _make_band(1.0)
    band64 = _make_band(1.0 / 64.0)
    bandf = _make_band_full0(1.0)
    band64f = _make_band_full0(1.0 / 64.0)
    in_maps = []
    for k in range(NCORES):
        bidx, c0 = k // 2, 3 * (k % 2)
        in_maps.append(
            {
                "a": np.ascontiguousarray(outputs[bidx, c0 : c0 + 3], dtype=np.float32),
                "b": np.ascontiguousarray(labels[bidx], dtype=np.float32),
                "band": band,
                "band64": band64,
                "bandf": bandf,
                "band64f": band64f,
            }
        )
    return in_maps


def combine(per_core_sums):
    loss_bc = np.zeros((4, 6), np.float64)
    for k in range(NCORES):
        bidx, c0 = k // 2, 3 * (k % 2)
        loss_bc[bidx, c0 : c0 + 3] = 1.0 - np.asarray(
            per_core_sums[k], np.float64
        ) / NPIX
    loss_bc32 = loss_bc.astype(np.float32)
    scalar = np.array(loss_bc.mean(), dtype=np.float32)
    return scalar, loss_bc32


def kernel(outputs, labels):
    from concourse.bass_utils import run_bass_kernel_spmd

    nc = _build()
    in_maps = make_in_maps(np.asarray(outputs), np.asarray(labels))
    res = run_bass_kernel_spmd(nc, in_maps, list(range(NCORES)))
    sums = [res.results[k]["out"][0, :3] for k in range(NCORES)]
    return combine(sums)
